# revision 39
# baseline (speedup 1.0000x reference)
"""Trainium2 Bass kernel for nn_Net_66975720014255 (gnn_message_passing).

Sharding: data-parallel over batch B=32 across 8 NeuronCores (4 batches per
core); adjacency and all weights replicated. No collectives.

v4: fp8-e4m3 DoubleRow adjacency hops (K=256/pass, 7 uniform supers with
zero-padded K, measured rel-err 1.7e-2 < 2e-2); bf16 everywhere else with
f32 PSUM accumulate.  Data movement: one merged DMA per batch for xp and
for the DR-packed windows, adjacency in 4 column-chunk DMAs on the gpsimd
queue, no input DMAs on the ACT engine (it is the busiest compute engine).
First two batches' tconv+residual run up front to fill the DMA head bubble;
the epilogue ladder is spread over DVE/ACT/GPSIMD so the last batch's tail
stays short.

Per-core device program (C=40, T=12, N=800, R=11):
  - tconv gates:  banded block matrix Wbig [480,440] (host-built) as lhsT,
                  rhs = xp tiles; tanh/sigmoid on ACT; product on DVE
  - hop0:         lhsT = wxt_dr fp8 [128,2,512] (DR pairs rows k,k+128),
                  rhs = adj_dr fp8; 7 uniform DoubleRow passes, PSUM acc
  - mix1:         per <=128-node chunk: lhsT = hop0 rows (bf16), rhs =
                  blockdiag(W1^T) -> transposed output relu-cast to fp8
                  directly into the DR-paired h1 tiles
  - hop1:         lhsT = h1_dr fp8, rhs = adj_dr[:, 800:]
  - mix2:         lhsT = blockdiag(W2^T) bf16, rhs = h2 rows bf16
  - skip/resid:   banded block matrices bf16, BN_SCALE folded on host
Embedding adds, adj=relu(nv1@nv2), fp8/bf16 quantization, weight reshaping,
BN folding: host numpy.
"""

import sys

if '/opt/trn_rl_repo' not in sys.path:
    sys.path.insert(0, '/opt/trn_rl_repo')

import numpy as np
import ml_dtypes

import concourse.bass as bass  # noqa: F401
import concourse.tile as tile
from concourse import bacc, mybir
from concourse.bass_utils import run_bass_kernel_spmd

# ----- problem constants (hardcoded per contract) -----
B, C, T, N = 32, 40, 12, 800
R = T - 1                    # 11
N2 = 2 * N                   # 1600
NCORES = 8
BL = B // NCORES             # 4 local batches per core
BN_SCALE = float(1.0 / np.sqrt(1.0 + 1e-5))

Q = T * C                    # 480 rows (t,c) per batch (non-T layout)
RQ = R * C                   # 440 rows (r,c) per batch
SQ = 12 * C                  # 480 skip rows (s,c) per batch

M_BLOCKS = [(0, 120), (120, 120), (240, 120), (360, 80)]          # (r,c) row blocks
NSUP = 7                     # DR supers: K=256 each; super 6 zero-padded past 1600
CH800 = [(0, 400), (400, 400)]
CH1600 = [(0, 400), (400, 400), (800, 400), (1200, 400)]
KROWS = [120, 120, 120, 80]  # wskip/oraw K block sizes

F32 = mybir.dt.float32
BF16 = mybir.dt.bfloat16
FP8 = mybir.dt.float8e4
DR = mybir.MatmulPerfMode.DoubleRow

_np_bf16 = ml_dtypes.bfloat16
_np_fp8 = ml_dtypes.float8_e4m3


# ---------------------------------------------------------------------------
# host-side preparation (pure numpy)
# ---------------------------------------------------------------------------

def _dr_pack_lhs(mat):
    """[1600, 440] -> DR-paired weights [128, NSUP, 2, 512].

    Slot i of super kk holds rows 256*kk + 128*i + p; the 440 (r,c) columns
    are padded out to m-blocks at 128-aligned offsets so every DoubleRow
    lhsT slice lands on a 16B boundary.  K is zero-padded from 1600 to
    NSUP*256 so the whole contraction is uniform DoubleRow passes.
    """
    padded = np.zeros((NSUP * 256, 512), np.float32)
    for j, (mo, ms) in enumerate(M_BLOCKS):
        padded[:N2, 128 * j:128 * j + ms] = mat[:, mo:mo + ms]
    q = padded.astype(_np_fp8)
    dr = np.zeros((128, NSUP, 2, 512), _np_fp8)
    for kk in range(NSUP):
        for i in range(2):
            base = 256 * kk + 128 * i
            dr[:, kk, i, :] = q[base:base + 128, :]
    return np.ascontiguousarray(dr)


def _prep_weights(inp):
    f32 = np.float32
    nv1, nv2 = np.asarray(inp['nv1'], f32), np.asarray(inp['nv2'], f32)
    adj = np.maximum(f32(0), nv1 @ nv2)                       # (1600,1600)

    adj_q = np.zeros((NSUP * 256, N2), _np_fp8)
    adj_q[:N2] = adj.astype(_np_fp8)
    adj_dr = np.zeros((128, NSUP, 2, N2), _np_fp8)
    for kk in range(NSUP):
        for i in range(2):
            base = 256 * kk + 128 * i
            adj_dr[:, kk, i, :] = adj_q[base:base + 128, :]

    def wbig(W):                                  # -> [120, 4, 440]
        Wb = np.zeros((Q, RQ), f32)
        W0, W1 = np.asarray(W[:, :, 0], f32), np.asarray(W[:, :, 1], f32)
        for r in range(R):
            Wb[r * C:(r + 1) * C, r * C:(r + 1) * C] = W0.T          # t == r
            Wb[(r + 1) * C:(r + 2) * C, r * C:(r + 1) * C] = W1.T    # t == r+1
        return np.ascontiguousarray(
            Wb.reshape(4, 120, RQ).transpose(1, 0, 2).astype(_np_bf16))

    def blkdiag3(A):                                          # A is (c, d)
        M = np.zeros((120, 120), f32)
        for j in range(3):
            M[j * C:(j + 1) * C, j * C:(j + 1) * C] = A
        return M.astype(_np_bf16)

    wmix1 = blkdiag3(np.asarray(inp['W_gcn'][0], f32).T)
    wmix2 = blkdiag3(np.asarray(inp['W_gcn'][1], f32).T)

    eye = np.eye(C, dtype=f32)
    wskip = np.zeros((RQ, SQ), f32)
    Ws = np.asarray(inp['W_skip'], f32) * BN_SCALE            # (12, 11)
    for s in range(12):
        for r in range(R):
            wskip[r * C:(r + 1) * C, s * C:(s + 1) * C] = Ws[s, r] * eye
    wskip_r = np.zeros((120, 4, SQ), f32)                     # [120, 4, 480]
    for kt, (o, sz) in enumerate(zip([0, 120, 240, 360], KROWS)):
        wskip_r[0:sz, kt, :] = wskip[o:o + sz, :]

    wres = np.zeros((Q, RQ), f32)
    Wr = np.asarray(inp['W_res'], f32) * BN_SCALE             # (11, 12)
    for t in range(T):
        for r in range(R):
            wres[t * C:(t + 1) * C, r * C:(r + 1) * C] = Wr[r, t] * eye
    wres_r = np.ascontiguousarray(
        wres.reshape(4, 120, RQ).transpose(1, 0, 2).astype(_np_bf16))

    biasfg = np.stack([np.tile(np.asarray(inp['b_f'], f32), 3),
                       np.tile(np.asarray(inp['b_g'], f32), 3)], axis=1)

    bres = np.asarray(inp['b_res'], f32) * BN_SCALE           # (11,)
    bres_tile = np.zeros((120, 1), f32)
    for p in range(120):
        r = p // C
        bres_tile[p, 0] = bres[r] if r < R else 0.0

    return dict(adj_dr=np.ascontiguousarray(adj_dr),
                wbig_f=wbig(np.asarray(inp['W_f'])),
                wbig_g=wbig(np.asarray(inp['W_g'])),
                wmix1=wmix1, wmix2=wmix2,
                wskip=np.ascontiguousarray(wskip_r.astype(_np_bf16)),
                wres=wres_r,
                biasfg=np.ascontiguousarray(biasfg),
                bres_tile=bres_tile, has_bres=bool(np.any(bres)))


def _prep_data(inp):
    f32 = np.float32
    x = np.asarray(inp['x'], f32) + np.asarray(inp['t_emb'], f32) \
        + np.asarray(inp['s_emb'], f32)                        # (B,C,T,N)
    xp = np.ascontiguousarray(x.transpose(0, 2, 1, 3)).reshape(B, Q, N)
    xpt = np.ascontiguousarray(x.transpose(0, 3, 2, 1)).reshape(B, N, Q)
    # windowed transpose: rows k in [0,800) -> x'[c, r, k]; k in [800,1600) ->
    # x'[c, r+1, k-800]; cols (r, c) = first 440 resp. last 440 of (t, c)
    wxt = np.concatenate([xpt[:, :, :RQ], xpt[:, :, C:]], axis=1)  # (B, 1600, 440)
    xp_r = np.ascontiguousarray(
        xp.reshape(B, 4, 120, N).transpose(0, 2, 1, 3).astype(_np_bf16))
    xp_cores, wdr_cores = [], []
    for i in range(NCORES):
        xp_cores.append(np.ascontiguousarray(xp_r[i * BL:(i + 1) * BL]))
        wdr_cores.append(np.stack(
            [_dr_pack_lhs(wxt[i * BL + b]) for b in range(BL)]))
    return xp_cores, wdr_cores       # (BL,120,4,800) bf16, (BL,128,NSUP,2,512)


# ---------------------------------------------------------------------------
# device program
# ---------------------------------------------------------------------------

def _build_program(has_bres):
    nc = bacc.Bacc("TRN2", target_bir_lowering=False, debug=False,
                   enable_asserts=False, num_devices=NCORES)

    xp_d = nc.dram_tensor("xp", [BL, 120, 4, N], BF16, kind="ExternalInput").ap()
    wdr_d = nc.dram_tensor("wdr", [BL, 128, NSUP, 2, 512], FP8,
                           kind="ExternalInput").ap()
    adjdr_d = nc.dram_tensor("adj_dr", [128, NSUP, 2, N2], FP8,
                             kind="ExternalInput").ap()
    wbigf_d = nc.dram_tensor("wbig_f", [120, 4, RQ], BF16, kind="ExternalInput").ap()
    wbigg_d = nc.dram_tensor("wbig_g", [120, 4, RQ], BF16, kind="ExternalInput").ap()
    wmix1_d = nc.dram_tensor("wmix1", [120, 120], BF16, kind="ExternalInput").ap()
    wmix2_d = nc.dram_tensor("wmix2", [120, 120], BF16, kind="ExternalInput").ap()
    wskip_d = nc.dram_tensor("wskip", [120, 4, SQ], BF16, kind="ExternalInput").ap()
    wres_d = nc.dram_tensor("wres", [120, 4, RQ], BF16, kind="ExternalInput").ap()
    biasfg_d = nc.dram_tensor("biasfg", [120, 2], F32, kind="ExternalInput").ap()
    bres_d = nc.dram_tensor("bres", [120, 1], F32, kind="ExternalInput").ap()
    # output rows per batch: 0:440 final (r,c), 440:920 skip (s,c)
    out_d = nc.dram_tensor("out", [BL, 920, N], F32, kind="ExternalOutput").ap()

    with tile.TileContext(nc) as tc:
        _emit(nc, tc, xp_d, wdr_d, adjdr_d, wbigf_d, wbigg_d,
              wmix1_d, wmix2_d, wskip_d, wres_d, biasfg_d, bres_d, out_d,
              has_bres)
    nc.compile()
    return nc


def _emit(nc, tc, xp_d, wdr_d, adjdr_d, wbigf_d, wbigg_d,
          wmix1_d, wmix2_d, wskip_d, wres_d, biasfg_d, bres_d, out_d,
          has_bres):
    from contextlib import ExitStack
    AF = mybir.ActivationFunctionType
    ALU = mybir.AluOpType
    ctx = ExitStack()
    with ctx:
        const = ctx.enter_context(tc.tile_pool(name="const", bufs=1))
        xp_p = ctx.enter_context(tc.tile_pool(name="xp", bufs=4))
        wdr_p = ctx.enter_context(tc.tile_pool(name="wdr", bufs=4))
        dres_p = ctx.enter_context(tc.tile_pool(name="dres", bufs=2))
        res_p = ctx.enter_context(tc.tile_pool(name="res", bufs=1))
        hop0_p = ctx.enter_context(tc.tile_pool(name="hop0sb", bufs=4))
        h1t_p = ctx.enter_context(tc.tile_pool(name="h1t", bufs=1))
        h2_p = ctx.enter_context(tc.tile_pool(name="h2sb", bufs=4))
        oraw_p = ctx.enter_context(tc.tile_pool(name="oraw", bufs=2))
        tmp_p = ctx.enter_context(tc.tile_pool(name="tmp", bufs=2))
        fin_p = ctx.enter_context(tc.tile_pool(name="fin", bufs=3))
        psA = ctx.enter_context(tc.tile_pool(name="psA", bufs=6, space="PSUM"))
        psB = ctx.enter_context(tc.tile_pool(name="psB", bufs=2, space="PSUM"))

        # ---- input DMA plan ----
        # All hw-dge DMAs share one counting semaphore: a consumer waits for
        # every DMA emitted before it.  So DMAs are emitted in need-order,
        # interleaved with the compute that unblocks, and the adjacency goes
        # on the gpsimd queue (its own semaphore) so nothing waits on it.
        # Output stores also go to gpsimd (adjacency is done by then).
        adj_sb = const.tile([128, NSUP, 2, N2], FP8, name="adj")
        for (co, cs) in CH1600:
            nc.gpsimd.dma_start(adj_sb[:, :, :, co:co + cs],
                                adjdr_d[:, :, :, co:co + cs])

        biasfg_sb = const.tile([120, 2], F32, name="biasfg")
        nc.sync.dma_start(biasfg_sb[:], biasfg_d[:])
        wbigf_sb = const.tile([120, 4, RQ], BF16, name="wbigf")
        nc.sync.dma_start(wbigf_sb[:], wbigf_d[:])
        wbigg_sb = const.tile([120, 4, RQ], BF16, name="wbigg")
        nc.scalar.dma_start(wbigg_sb[:], wbigg_d[:])
        wbig_sb = {"f": wbigf_sb, "g": wbigg_sb}

        def load_xp(b):
            xp_sb = xp_p.tile([120, 4, N], BF16, name=f"xp{b}", tag="xp", bufs=4)
            nc.sync.dma_start(xp_sb[:, 0:2, :], xp_d[b, :, 0:2, :])
            nc.scalar.dma_start(xp_sb[:, 2:4, :], xp_d[b, :, 2:4, :])
            return xp_sb

        def load_wdr(b):
            wdr_sb = wdr_p.tile([128, NSUP, 2, 512], FP8, name=f"wdr{b}",
                                tag="wdr", bufs=4)
            for kk in range(NSUP):
                eng = nc.sync if kk % 2 == 0 else nc.scalar
                eng.dma_start(wdr_sb[:, kk], wdr_d[b, :, kk])
            return wdr_sb

        def load_b(b):
            return load_xp(b), load_wdr(b)

        loads = [None]
        xp0_sb = load_xp(0)
        wmix1_sb = const.tile([120, 120], BF16, name="wmix1")
        nc.sync.dma_start(wmix1_sb[:], wmix1_d[:])
        wmix2_sb = const.tile([120, 120], BF16, name="wmix2")
        nc.scalar.dma_start(wmix2_sb[:], wmix2_d[:])
        # deferred-const tiles; DMAs are emitted later, in need-order
        wskip_sb = const.tile([120, 4, SQ], BF16, name="wskip")
        wres_sb = const.tile([120, 4, RQ], BF16, name="wres")
        bres_sb = const.tile([120, 1], F32, name="bres_t")

        def load_consts1():
            nc.sync.dma_start(wres_sb[:], wres_d[:])
            nc.sync.dma_start(wskip_sb[:], wskip_d[:])
            nc.sync.dma_start(bres_sb[:], bres_d[:])


        def tconv_b(b, xp_sb):
            dres_sb = []
            for m, (mo, ms) in enumerate(M_BLOCKS):
                dr = dres_p.tile([120, N], BF16, name=f"dres{m}", tag=f"dres{m}",
                                 bufs=2)
                dres_sb.append(dr)
                kts = [m] if m == 3 else [m, m + 1]
                gate_sb = {}
                for gi, gname in enumerate(("f", "g")):
                    for (co, cs) in CH800:
                        ps = psA.tile([120, 400], F32, name="tc_ps", tag="psA")
                        for j, kt in enumerate(kts):
                            nc.tensor.matmul(
                                ps[0:ms, :],
                                wbig_sb[gname][:, kt, mo:mo + ms],
                                xp_sb[:, kt, co:co + cs],
                                start=(j == 0), stop=(j == len(kts) - 1))
                        g = tmp_p.tile([120, 400], BF16, name=f"g{gname}",
                                       tag=f"gate{gname}{co}", bufs=2)
                        nc.scalar.activation(
                            g[0:ms, :], ps[0:ms, :],
                            AF.Tanh if gname == "f" else AF.Sigmoid,
                            bias=biasfg_sb[0:ms, gi:gi + 1])
                        gate_sb[(gname, co)] = g
                for (co, cs) in CH800:
                    nc.vector.tensor_mul(dr[0:ms, co:co + cs],
                                         gate_sb[("f", co)][0:ms, :],
                                         gate_sb[("g", co)][0:ms, :])
            return dres_sb

        def res_b(b, xp_sb):
            # residual = data @ wres, parked in SBUF bf16; fills the head
            # bubble while adj/wxt stream in, and trims the per-batch tail
            res_sb = []
            for m, (mo, ms) in enumerate(M_BLOCKS):
                rs = res_p.tile([120, N], BF16, name=f"res{m}",
                                tag=f"res{b}_{m}", bufs=1)
                res_sb.append(rs)
                for (co, cs) in CH800:
                    ps = psA.tile([120, 400], F32, name="rs_ps", tag="psA")
                    for kt in range(4):
                        nc.tensor.matmul(
                            ps[0:ms, :],
                            wres_sb[:, kt, mo:mo + ms],
                            xp_sb[:, kt, co:co + cs],
                            start=(kt == 0), stop=(kt == 3))
                    nc.vector.tensor_copy(rs[0:ms, co:co + cs], ps[0:ms, :])
            return res_sb

        def mix2_m(m, h2_tiles, oraw_sb, dres_sb):
            mo, ms = M_BLOCKS[m]
            h2 = h2_tiles[m]
            orw = oraw_sb[m]
            for (co, cs) in CH800:
                ps = psA.tile([120, 400], F32, name="b2_ps", tag="psA")
                nc.tensor.matmul(ps[0:ms, :],
                                 wmix2_sb[0:ms, 0:ms],
                                 h2[0:ms, co:co + cs],
                                 start=True, stop=True)
                rl = tmp_p.tile([120, 400], BF16, name="rl", tag=f"rl{co}", bufs=2)
                nc.scalar.activation(rl[0:ms, :], ps[0:ms, :], AF.Relu)
                nc.vector.tensor_add(orw[0:ms, co:co + cs], rl[0:ms, :],
                                     dres_sb[m][0:ms, co:co + cs])

        def hops_b(b, wdr_sb, dres_sb, last=False):
            # hop0 (fp8 DoubleRow) -> h0 bf16 rows; mix1 -> h1 fp8 DR-paired
            h1dr_sb = []
            for kk in range(NSUP):
                h1dr_sb.append(h1t_p.tile([128, 2, 512], FP8, name=f"h1dr{kk}",
                                          tag=f"h1dr{kk}", bufs=1))
            # super 6 is only partially written by mix1 (rows past 1600 are
            # K-padding); zero it so stale SBUF bytes can't inject NaN*0
            nc.gpsimd.memset(h1dr_sb[NSUP - 1][:], 0.0)
            h0_tiles = []
            for m, (mo, ms) in enumerate(M_BLOCKS):
                h0 = hop0_p.tile([120, N2], BF16, name="h0", tag="h0", bufs=4)
                h0_tiles.append(h0)
                for (co, cs) in CH1600:
                    ps = psA.tile([120, 400], F32, name="h0_ps", tag="psA")
                    for kk in range(NSUP):
                        nc.tensor.matmul(
                            ps[0:ms, :],
                            wdr_sb[:, kk, :, 128 * m:128 * m + ms],
                            adj_sb[:, kk, :, co:co + cs],
                            start=(kk == 0), stop=(kk == NSUP - 1), perf_mode=DR)
                    nc.vector.tensor_copy(h0[0:ms, co:co + cs], ps[0:ms, :])
            for st in range(13):                       # node s-tiles of 128/64
                s = 64 if st == 12 else 128
                o = st * 128
                for m, (mo, ms) in enumerate(M_BLOCKS):
                    h0 = h0_tiles[m]
                    bp = psB.tile([128, 120], F32, name="b1_ps", tag="psB")
                    nc.tensor.matmul(bp[0:s, 0:ms],
                                     h0[0:ms, o:o + s],
                                     wmix1_sb[0:ms, 0:ms],
                                     start=True, stop=True)
                    dst = h1dr_sb[st // 2][0:s, st % 2, 128 * m:128 * m + ms]
                    nc.vector.tensor_relu(dst, bp[0:s, 0:ms])
            # hop1 (fp8 DoubleRow) + mix2 + data_res add -> out_raw
            oraw_sb = []
            h2_tiles = []
            for m, (mo, ms) in enumerate(M_BLOCKS):
                orw = oraw_p.tile([120, N], BF16, name=f"oraw{m}", tag=f"oraw{m}",
                                  bufs=2)
                oraw_sb.append(orw)
                h2 = h2_p.tile([120, N], BF16, name="h2", tag="h2", bufs=4)
                h2_tiles.append(h2)
                for (co, cs) in CH800:
                    ps = psA.tile([120, 400], F32, name="h1_ps", tag="psA")
                    for kk in range(NSUP):
                        nc.tensor.matmul(
                            ps[0:ms, :],
                            h1dr_sb[kk][:, :, 128 * m:128 * m + ms],
                            adj_sb[:, kk, :, 800 + co:800 + co + cs],
                            start=(kk == 0), stop=(kk == NSUP - 1), perf_mode=DR)
                    nc.scalar.copy(h2[0:ms, co:co + cs], ps[0:ms, :])
            for m in range(4):
                mix2_m(m, h2_tiles, oraw_sb, dres_sb)
            return oraw_sb

        def epilogue_b(b, res_sb, xp_sb, oraw_sb):
            # skip -> out rows 440:920
            for sm in range(4):
                sk = fin_p.tile([120, N], F32, name="sk", tag="sk", bufs=3)
                for (co, cs) in CH800:
                    ps = psA.tile([120, 400], F32, name="sk_ps", tag="psA")
                    for kt in range(4):
                        nc.tensor.matmul(
                            ps[:, :],
                            wskip_sb[0:KROWS[kt], kt, sm * 120:(sm + 1) * 120],
                            oraw_sb[kt][0:KROWS[kt], co:co + cs],
                            start=(kt == 0), stop=(kt == 3))
                    nc.scalar.copy(sk[:, co:co + cs], ps[:, :])
                eng = (nc.gpsimd if b < BL - 1 else
                       [nc.gpsimd, nc.sync, nc.scalar, nc.gpsimd][sm])
                eng.dma_start(
                    out_d[b, RQ + sm * 120:RQ + (sm + 1) * 120, :], sk[:, :])
            # final combine -> out rows 0:440; residual either precomputed in
            # SBUF (head batches) or streamed through PSUM here
            for m, (mo, ms) in enumerate(M_BLOCKS):
                fin = fin_p.tile([120, N], F32, name="fin", tag="fin", bufs=3)
                for ci, (co, cs) in enumerate(CH800):
                    if res_sb is not None:
                        radd = res_sb[m][0:ms, co:co + cs]
                    else:
                        ps = psA.tile([120, 400], F32, name="rs_ps", tag="psA")
                        for kt in range(4):
                            nc.tensor.matmul(
                                ps[0:ms, :],
                                wres_sb[:, kt, mo:mo + ms],
                                xp_sb[:, kt, co:co + cs],
                                start=(kt == 0), stop=(kt == 3))
                        radd = ps[0:ms, :]
                    nc.vector.scalar_tensor_tensor(
                        fin[0:ms, co:co + cs], oraw_sb[m][0:ms, co:co + cs],
                        BN_SCALE, radd, op0=ALU.mult, op1=ALU.add)
                    if has_bres:
                        nc.vector.tensor_scalar_add(fin[0:ms, co:co + cs],
                                                    fin[0:ms, co:co + cs],
                                                    bres_sb[0:ms, :])
                eng = (nc.gpsimd if b < BL - 1 else
                       [nc.sync, nc.gpsimd, nc.scalar, nc.sync][m])
                eng.dma_start(out_d[b, mo:mo + ms, :], fin[0:ms, :])

        # pipeline: per-batch loads emitted lazily so the shared DMA semaphore
        # never makes early compute wait on later batches' bytes; residual of
        # the last batch parked in SBUF so its epilogue tail is matmul-free
        prev = None
        for b in range(BL):
            if b == 0:
                xp_sb = xp0_sb
                dres_sb = tconv_b(b, xp_sb)
                wdr_sb = load_wdr(0)     # emitted after tconv(0): the gate
                                         # matmuls don't wait on these bytes
            else:
                xp_sb, wdr_sb = load_b(b)
                dres_sb = tconv_b(b, xp_sb)
            if b == 1:
                load_consts1()
            res_sb = res_b(b, xp_sb) if b == 3 else None
            if prev is not None:
                epilogue_b(*prev)
            oraw_sb = hops_b(b, wdr_sb, dres_sb)
            prev = (b, res_sb, xp_sb, oraw_sb)
        epilogue_b(*prev)


_CACHE = {}


def kernel(**inputs):
    w = _prep_weights(inputs)
    xp_cores, wdr_cores = _prep_data(inputs)

    key = ("prog", w['has_bres'])
    if key not in _CACHE:
        _CACHE[key] = _build_program(has_bres=w['has_bres'])
    nc = _CACHE[key]

    in_maps = []
    for core in range(NCORES):
        in_maps.append({
            "xp": xp_cores[core],
            "wdr": wdr_cores[core],
            "adj_dr": w['adj_dr'],
            "wbig_f": w['wbig_f'],
            "wbig_g": w['wbig_g'],
            "wmix1": w['wmix1'],
            "wmix2": w['wmix2'],
            "wskip": w['wskip'],
            "wres": w['wres'],
            "biasfg": w['biasfg'],
            "bres": w['bres_tile'],
        })

    import os
    trace = bool(int(os.environ.get("KERNEL_TRACE", "0")))
    res = run_bass_kernel_spmd(nc, in_maps, core_ids=list(range(NCORES)),
                               trace=trace)
    kernel.last_result = res
    outs = [r["out"] for r in res.results]            # each (BL, 920, 800)
    full = np.concatenate(outs, axis=0)               # (32, 920, 800)
    full = full.reshape(B, 23, C, N).transpose(0, 2, 1, 3)   # (B, C, 23, N)
    return np.ascontiguousarray(full)
